# revision 16
# baseline (speedup 1.0000x reference)
"""Class-conditional BatchNorm2d (eval path, alpha=0.5) on 8 Trainium2 cores.

Strategy (data-parallel over batch, per the sharding hint):
  - Each of the 8 cores gets 16 of the 128 samples; the small stat
    tables are replicated — digested on the host into per-sample
    per-channel scale/shift (a [C, 2*BS] f32 table, 16 KiB per core):
        scale[b,c] = weight[c] / sqrt(var[b,c] + eps)
        shift[b,c] = bias[c] - mean[b,c] * scale[b,c]
    where mean/var interpolate global and class running stats
    (alpha=0.5, class row gathered by label). This is 0.25% of the
    arithmetic; the 205 MiB streaming multiply-add stays on device.
  - The bulk x/out traffic moves as fp16 (correctness gate is 2e-2
    rel; fp16 quantization contributes ~1e-3), halving HBM bytes vs
    f32. The host transposes each core's x to [C, BS*HW] so every
    tile's partition line is contiguous (2-sample tiles: 12544 B, the
    per-engine DMA packet sweet spot).
  - Device pipeline, per core. Tile sizes [1,1,2,2,2,2,2,2,1,1]
    samples: small head tiles let the first stores issue ~3us earlier
    (longer mixed read/write phase — the chip sustains ~3.1 TB/s
    mixed vs ~2.55 TB/s pure-write), small tail tiles shrink the
    serial last-load -> compute -> last-store drain.
      sync (SP) HWDGE ring:    loads, in tile order
      scalar (Act) HWDGE ring: the scale/shift table, then stores
      DVE: per sample one fused tensor_scalar (x*scale + shift) in
           2x fp16 mode with f32 per-partition scalars, in place
    Loads and stores alternate in trace order (L0,S0,L1,S1,...) so
    each of the ~8 rotating HWDGE semaphores is recycled onto a DMA
    whose predecessor finished long ago — no issue-pipeline stalls.
  - DGE descriptor spray: contiguous chunks of ceil(n/16) descriptors
    round-robin from engine 0, so a 120-partition DMA skips engine 15
    entirely. Engine 15 hosts the DMA queue rings and runs ~60ns/pkt
    slower; splitting two 2-sample loads as [0:120]+[120:128] takes
    ~12.5% of bytes off it and equalizes per-engine finish time.
"""

import numpy as np
from contextlib import ExitStack

import concourse.bacc as bacc
import concourse.tile as tile
from concourse import mybir
from concourse.bass_utils import run_bass_kernel_spmd

B, C, H, W = 128, 128, 56, 56
HW = H * W
NCORES = 8
BS = B // NCORES  # 16 samples per core
EPS = 1e-5
ALPHA = 0.5

# per-tile sample counts: small head tiles (early stores), small tail
# tiles (short drain), 12544 B lines in the middle
SIZES = [1, 1, 2, 2, 2, 2, 2, 2, 1, 1]
OFFS = np.cumsum([0] + SIZES[:-1]).tolist()
SPLIT_TILES = (2, 3)  # 2-sample tiles load-split [0:120]+[120:128]
assert sum(SIZES) == BS

F32 = mybir.dt.float32
F16 = mybir.dt.float16

_CACHED_NC = None


def _build_nc():
    nc = bacc.Bacc(
        "TRN2",
        debug=False,
        enable_asserts=False,
        target_bir_lowering=False,
        num_devices=NCORES,
    )

    # x transposed on host to [C, BS*HW] fp16: columns s*HW..(s+1)*HW
    # hold sample s for channel (partition) c — any tile of consecutive
    # samples is a contiguous per-partition run
    x_d = nc.dram_tensor("x", [C, BS * HW], F16, kind="ExternalInput")
    # host-digested [scale | shift] per sample: columns 0..BS-1 scale,
    # BS..2*BS-1 shift, partition = channel
    ss_d = nc.dram_tensor("ss", [C, 2 * BS], F32, kind="ExternalInput")
    out_d = nc.dram_tensor("out", [C, BS * HW], F16, kind="ExternalOutput")

    with tile.TileContext(nc) as tc, ExitStack() as ctx:
        const = ctx.enter_context(tc.tile_pool(name="const", bufs=1))
        data = ctx.enter_context(tc.tile_pool(name="data", bufs=len(SIZES)))

        # scale/shift table rides the scalar ring (no store to issue for
        # a while) so the sync ring's first instruction is load 0
        ss_sb = const.tile([C, 2 * BS], F32)
        nc.scalar.dma_start(ss_sb[:], ss_d.ap())
        scale_col = ss_sb[:, 0:BS]
        shift_col = ss_sb[:, BS : 2 * BS]

        for t, n in enumerate(SIZES):
            c0 = OFFS[t] * HW
            cn = n * HW
            xt = data.tile([C, cn], F16, name="xt")
            src = x_d.ap()[:, c0 : c0 + cn]
            if t in SPLIT_TILES:
                nc.sync.dma_start(xt[0:120, :], src[0:120])
                nc.sync.dma_start(xt[120:C, :], src[120:C])
            else:
                nc.sync.dma_start(xt[:], src)
            for h in range(n):
                s = OFFS[t] + h
                nc.vector.tensor_scalar(
                    xt[:, h * HW : (h + 1) * HW],
                    xt[:, h * HW : (h + 1) * HW],
                    scale_col[:, s : s + 1],
                    shift_col[:, s : s + 1],
                    mybir.AluOpType.mult,
                    mybir.AluOpType.add,
                )
            nc.scalar.dma_start(out_d.ap()[:, c0 : c0 + cn], xt[:])

    nc.compile()
    return nc


def _get_nc():
    global _CACHED_NC
    if _CACHED_NC is None:
        _CACHED_NC = _build_nc()
    return _CACHED_NC


def _make_in_maps(inputs):
    x = np.asarray(inputs["x"]).astype(np.float16).reshape(B, C, HW)
    labels = np.asarray(inputs["labels"]).astype(np.int64)
    weight = np.asarray(inputs["weight"], dtype=np.float32)
    bias = np.asarray(inputs["bias"], dtype=np.float32)
    gmean = np.asarray(inputs["global_running_mean"], dtype=np.float32)
    gvar = np.asarray(inputs["global_running_var"], dtype=np.float32)
    cmean = np.asarray(inputs["class_running_mean"], dtype=np.float32)
    cvar = np.asarray(inputs["class_running_var"], dtype=np.float32)

    # per-sample stats, same formula as the reference (f32)
    mean = (1.0 - ALPHA) * gmean[None, :] + ALPHA * cmean[labels]  # [B, C]
    var = (1.0 - ALPHA) * gvar[None, :] + ALPHA * cvar[labels]
    scale = weight[None, :] / np.sqrt(var + EPS)
    shift = bias[None, :] - mean * scale

    in_maps = []
    for i in range(NCORES):
        sl = slice(i * BS, (i + 1) * BS)
        # [BS, C, HW] -> [C, BS*HW]: sample-major columns per channel
        xr = np.ascontiguousarray(
            x[sl].transpose(1, 0, 2)
        ).reshape(C, BS * HW)
        ss = np.ascontiguousarray(
            np.concatenate([scale[sl].T, shift[sl].T], axis=1)
        )  # [C, 2*BS]
        in_maps.append({"x": xr, "ss": ss})
    return in_maps


def _run(inputs, trace=False, **kwargs):
    nc = _get_nc()
    in_maps = _make_in_maps(inputs)
    return run_bass_kernel_spmd(
        nc, in_maps, list(range(NCORES)), trace=trace, **kwargs
    )


def _gather(res) -> np.ndarray:
    out = np.empty((B, C, H, W), dtype=np.float32)
    for i in range(NCORES):
        o = np.asarray(res.results[i]["out"]).reshape(C, BS, HW)
        out[i * BS : (i + 1) * BS] = (
            o.transpose(1, 0, 2).reshape(BS, C, H, W).astype(np.float32)
        )
    return out


def kernel(**inputs) -> np.ndarray:
    res = _run(inputs, trace=False)
    return _gather(res)
